# revision 37
# baseline (speedup 1.0000x reference)
"""Trainium2 Bass kernel for a multi-head-attention block (B,C,N,D = 8,4,1024,96;
H=3 heads, dk=dv=32; softmax over the QUERY axis; residual + LayerNorm).

Sharding: pure data-parallel over batch B across 8 NeuronCores (one batch
element per core, C=4 channel-slices each, no collectives).

Per (b, c) on-device computation:
  - transpose inputs X -> X.T [d, tok] via single-pass f32r PE
    matmul-with-identity
  - Q_dT/K_dT projections in [e, tok] layout (heads stacked on partitions,
    f32r single-pass), V in natural [tok, e] layout
  - scores S_T[k, q] = K_chunk.T @ Q (f32r, contraction d=32); the three
    heads' matmuls sit at row-groups 0/32/64 and are emitted back-to-back so
    they run concurrently in the PE array
  - exp on ScalarE (the pacing engine) with fused 1/sqrt(dk) scale, bf16
    output, accum_out -> softmax denominators; normalization folded into
    V' = V * (1/sums) (bf16)
  - context matmuls in bf16 at col-groups 0/32/64 (concurrent), accumulated
    over chunks in PSUM; channel c's scores/exp interleave with channel c-1's
    context matmuls to keep the tensor engine dense (HAM warm)
  - fc (f32r) + residual + LayerNorm; rsqrt(var+eps) via bit-hack + 2 Newton
    iterations on the vector engine (no extra ScalarE work, no table switch)
"""

from contextlib import ExitStack

import numpy as np

import concourse.bass as bass
import concourse.tile as tile
from concourse import bacc, mybir
from concourse.bass_utils import run_bass_kernel_spmd

F32 = mybir.dt.float32
BF16 = mybir.dt.bfloat16
F32R = mybir.dt.float32r
I32 = mybir.dt.int32
A = mybir.AluOpType

B, C, N, D = 8, 4, 1024, 96
H, DK, DV = 3, 32, 32
P = 128               # partition size / token chunk
NCHUNK = N // P       # 8
QT = 512              # matmul free-dim limit into one PSUM bank (f32)
SCALE = 1.0 / np.sqrt(DK)
EPS = 1e-5

_CACHE = {}


def _emit(nc, tc, ctx, apply_affine):
    xq_d = nc.dram_tensor("xq", [C, N, D], F32, kind="ExternalInput").ap()
    xk_d = nc.dram_tensor("xk", [C, N, D], F32, kind="ExternalInput").ap()
    xv_d = nc.dram_tensor("xv", [C, N, D], F32, kind="ExternalInput").ap()
    wq_d = nc.dram_tensor("wq", [D, D], F32, kind="ExternalInput").ap()
    wk_d = nc.dram_tensor("wk", [D, D], F32, kind="ExternalInput").ap()
    wv_d = nc.dram_tensor("wv", [D, D], F32, kind="ExternalInput").ap()
    wfc_d = nc.dram_tensor("wfc", [D, D], F32, kind="ExternalInput").ap()
    gam_d = nc.dram_tensor("gam", [D], F32, kind="ExternalInput").ap()
    bet_d = nc.dram_tensor("bet", [D], F32, kind="ExternalInput").ap()
    out_d = nc.dram_tensor("out", [C, N, D], F32, kind="ExternalOutput").ap()

    ident_d = nc.inline_tensor(np.eye(P, dtype=np.float32), name="ident")

    const = ctx.enter_context(tc.tile_pool(name="const", bufs=1))
    pc = ctx.enter_context(tc.tile_pool(name="perc", bufs=2))
    # one merged work pool: 3 slots x [128, 1024] f32 = 6 PSUM banks; serves
    # the score regions (3 concurrent head-regions per chunk) and all the
    # phase-1/fc utility matmul outputs.  ctx pool: remaining 2 banks.
    w_psum = ctx.enter_context(tc.tile_pool(name="w_psum", bufs=3, space="PSUM"))
    ctx_psum = ctx.enter_context(tc.tile_pool(name="ctx_psum", bufs=1, space="PSUM"))

    ident = const.tile([P, P], F32R)
    nc.gpsimd.dma_start(out=ident, in_=ident_d.ap())

    # weights: load natural, transpose on PE (out = W.T since out = lhsT.T @ I)
    wts = {}
    for nm, wd in (("wq", wq_d), ("wk", wk_d), ("wv", wv_d), ("wfc", wfc_d)):
        w_nat = const.tile([D, D], F32R, name=f"{nm}_nat", tag="w_nat")
        nc.gpsimd.dma_start(out=w_nat, in_=wd)
        w_ps = w_psum.tile([D, D], F32, name=f"{nm}_ps", tag="w")
        nc.tensor.matmul(w_ps, lhsT=w_nat, rhs=ident[:D, :D], start=True, stop=True)
        w_t = const.tile([D, D], F32R, name=f"{nm}T", tag=f"{nm}T")
        nc.vector.tensor_copy(out=w_t, in_=w_ps)
        wts[nm] = w_t

    gam_tile = bet_tile = None
    if apply_affine:
        gam_tile = const.tile([P, D], F32)
        bet_tile = const.tile([P, D], F32)
        for t, d_ap in ((gam_tile, gam_d), (bet_tile, bet_d)):
            bcast = bass.AP(tensor=d_ap.tensor, offset=d_ap.offset,
                            ap=[[0, P], d_ap.ap[0]])
            nc.gpsimd.dma_start(out=t, in_=bcast)

    st = {}

    def phase1(c):
        """loads, input transposes, Q/K/V projections for channel c"""
        xq_nat = pc.tile([P, NCHUNK, D], F32R, name=f"xq_nat{c}", tag="xq_nat",
                         bufs=3)
        xk_nat = pc.tile([P, NCHUNK, D], F32R, name=f"xk_nat{c}", tag="xk_nat",
                         bufs=1)
        xv_nat = pc.tile([P, NCHUNK, D], F32R, name=f"xv_nat{c}", tag="xv_nat",
                         bufs=1)
        for t, src in ((xq_nat, xq_d), (xk_nat, xk_d), (xv_nat, xv_d)):
            nc.gpsimd.dma_start(
                out=t, in_=src[c].rearrange("(i p) d -> p i d", p=P))

        xTs = {}
        for nm, src in (("q", xq_nat), ("k", xk_nat), ("v", xv_nat)):
            xT = pc.tile([D, N], F32R, name=f"x{nm}T{c}", tag=f"x{nm}T", bufs=1)
            for g in range(2):  # 4 chunk-transposes batched per PSUM bank
                tp_ps = w_psum.tile([D, 4 * P], F32, name=f"tp{nm}{c}{g}",
                                    tag="w")
                for j in range(4):
                    i = 4 * g + j
                    nc.tensor.matmul(tp_ps[:, j * P:(j + 1) * P],
                                     lhsT=src[:, i, :], rhs=ident,
                                     start=True, stop=True)
                nc.vector.tensor_copy(out=xT[:, g * 4 * P:(g + 1) * 4 * P],
                                      in_=tp_ps)
            xTs[nm] = xT

        qdT = pc.tile([D, N], F32R, name=f"qdT{c}", tag="qdT")
        kdT = pc.tile([D, N], F32R, name=f"kdT{c}", tag="kdT")
        for dst, w_t, xT in ((qdT, wts["wq"], xTs["q"]), (kdT, wts["wk"], xTs["k"])):
            for g in range(2):
                pr_ps = w_psum.tile([D, QT], F32, name=f"pr{c}{g}", tag="w")
                nc.tensor.matmul(pr_ps, lhsT=w_t,
                                 rhs=xT[:, g * QT:(g + 1) * QT],
                                 start=True, stop=True)
                nc.vector.tensor_copy(out=dst[:, g * QT:(g + 1) * QT], in_=pr_ps)

        v_nat = pc.tile([P, NCHUNK, D], F32, name=f"v_nat{c}", tag="v_nat")
        for g in range(2):
            v_ps = w_psum.tile([P, 4 * D], F32, name=f"vps{c}{g}", tag="w")
            for j in range(4):
                i = 4 * g + j
                nc.tensor.matmul(v_ps[:, j * D:(j + 1) * D],
                                 lhsT=xTs["v"][:, i * P:(i + 1) * P],
                                 rhs=wts["wv"], start=True, stop=True)
            nc.vector.tensor_copy(
                out=v_nat[:, 4 * g:4 * (g + 1), :].rearrange("p i d -> p (i d)"),
                in_=v_ps)

        ssum = pc.tile([P, H * NCHUNK], F32, name=f"ssum{c}", tag="ssum")
        sinv = pc.tile([P, H * NCHUNK], F32, name=f"sinv{c}", tag="sinv")
        e_all = pc.tile([P, H * NCHUNK, N], BF16, name=f"e{c}", tag="e")
        vsc_all = pc.tile([P, H * NCHUNK, DV], BF16, name=f"vsc{c}", tag="vsc")
        st[c] = dict(xq_nat=xq_nat, qdT=qdT, kdT=kdT, v_nat=v_nat,
                     ssum=ssum, sinv=sinv, e_all=e_all, vsc_all=vsc_all)

    def scores_exp(c, i):
        """S_T and exp for chunk i of channel c (ScalarE-paced).  The three
        heads' matmuls are adjacent at row-groups 0/32/64 -> concurrent."""
        s = st[c]
        s_regs = []
        for h in range(H):
            s_regs.append(w_psum.tile([P, N], F32, name=f"s{c}_{i}_{h}", tag="w"))
        for g in range(2):
            for h in range(H):
                hs = slice(DK * h, DK * (h + 1))
                nc.tensor.matmul(
                    s_regs[h][:, g * QT:(g + 1) * QT],
                    lhsT=s["kdT"][hs, i * P:(i + 1) * P],
                    rhs=s["qdT"][hs, g * QT:(g + 1) * QT],
                    start=True, stop=True)
        for h in range(H):
            j = i * H + h
            nc.scalar.activation(
                out=s["e_all"][:, j, :], in_=s_regs[h],
                func=mybir.ActivationFunctionType.Exp,
                scale=SCALE, accum_out=s["ssum"][:, j:j + 1])
        nc.vector.reciprocal(out=s["sinv"][:, i * H:(i + 1) * H],
                             in_=s["ssum"][:, i * H:(i + 1) * H])
        for h in range(H):
            hs = slice(DK * h, DK * (h + 1))
            j = i * H + h
            nc.vector.tensor_scalar_mul(
                out=s["vsc_all"][:, j, :], in0=s["v_nat"][:, i, hs],
                scalar1=s["sinv"][:, j:j + 1])

    def ctx_mm(c, i):
        """context accumulation for chunk i of channel c: bf16, three heads
        at col-groups 0/32/64, emitted adjacently -> concurrent."""
        s = st[c]
        for g in range(2):
            for h in range(H):
                hs = slice(DV * h, DV * (h + 1))
                j = i * H + h
                # per-head accumulation groups target disjoint 32-partition
                # col-groups of the same banks; the sim's zero-region tracker
                # can't see that, hence skip_group_check
                nc.tensor.matmul(
                    s["ctx_ps"][hs, g * QT:(g + 1) * QT],
                    lhsT=s["vsc_all"][:, j, :],
                    rhs=s["e_all"][:, j, g * QT:(g + 1) * QT],
                    start=(i == 0), stop=(i == NCHUNK - 1),
                    skip_group_check=True)

    def tail_a(c):
        """ctx copy-out for channel c (frees the ctx PSUM banks)"""
        s = st[c]
        ctxT = pc.tile([D, N], F32R, name=f"ctxT{c}", tag="ctxT")
        nc.vector.tensor_copy(out=ctxT, in_=s["ctx_ps"])
        s["ctxT"] = ctxT
        del s["ctx_ps"]

    def tail_b(c):
        """fc, residual, LayerNorm, store for channel c"""
        s = st[c]
        ctxT = s["ctxT"]
        t_all = pc.tile([P, NCHUNK, D], F32, name=f"tall{c}", tag="tall")
        sums = pc.tile([P, NCHUNK], F32, name=f"sums{c}", tag="sums")
        sumsq = pc.tile([P, NCHUNK], F32, name=f"sumsq{c}", tag="sumsq")
        sq_scr = pc.tile([P, NCHUNK, D], F32, name=f"sqscr{c}", tag="sqscr")
        for g in range(2):
            fc_ps = w_psum.tile([P, 4 * D], F32, name=f"fc{c}{g}", tag="w")
            for j in range(4):
                i = 4 * g + j
                nc.tensor.matmul(fc_ps[:, j * D:(j + 1) * D],
                                 lhsT=ctxT[:, i * P:(i + 1) * P],
                                 rhs=wts["wfc"], start=True, stop=True)
            for j in range(4):
                i = 4 * g + j
                nc.vector.scalar_tensor_tensor(
                    out=t_all[:, i, :], in0=fc_ps[:, j * D:(j + 1) * D],
                    scalar=1.0, in1=s["xq_nat"][:, i, :].bitcast(F32),
                    op0=A.mult, op1=A.add, accum_out=sums[:, i:i + 1])
                # (tensor_tensor_reduce wedges this runtime; scalar_tensor_tensor
                # with accum_out computes the same square-sums)
                nc.vector.scalar_tensor_tensor(
                    out=sq_scr[:, i, :], in0=t_all[:, i, :], scalar=1.0,
                    in1=t_all[:, i, :], op0=A.mult, op1=A.mult,
                    accum_out=sumsq[:, i:i + 1])

        # mean, var+eps
        mean = pc.tile([P, NCHUNK], F32, name=f"mean{c}", tag="mean")
        msq = pc.tile([P, NCHUNK], F32, name=f"msq{c}", tag="msq")
        var = pc.tile([P, NCHUNK], F32, name=f"var{c}", tag="var")
        nc.vector.tensor_scalar_mul(out=mean, in0=sums, scalar1=1.0 / D)
        nc.vector.tensor_mul(out=msq, in0=mean, in1=mean)
        nc.vector.scalar_tensor_tensor(
            out=var, in0=sumsq, scalar=1.0 / D, in1=msq,
            op0=A.mult, op1=A.subtract)
        nc.vector.tensor_scalar_add(out=var, in0=var, scalar1=EPS)
        # rstd = rsqrt(var) via bit-hack seed + 2 Newton iterations (DVE only)
        y = pc.tile([P, NCHUNK], F32, name=f"y{c}", tag="y")
        t1 = pc.tile([P, NCHUNK], F32, name=f"t1n{c}", tag="t1n")
        nc.vector.tensor_scalar(
            out=y.bitcast(I32), in0=var.bitcast(I32), scalar1=1,
            scalar2=None, op0=A.logical_shift_right)
        nc.vector.tensor_scalar(
            out=y.bitcast(I32), in0=y.bitcast(I32), scalar1=-1,
            scalar2=None, op0=A.bitwise_xor)
        nc.vector.tensor_scalar(
            out=y.bitcast(I32), in0=y.bitcast(I32), scalar1=0x5F3759E0,
            scalar2=None, op0=A.add)
        for _ in range(2):
            nc.vector.tensor_mul(out=t1, in0=y, in1=y)
            nc.vector.tensor_mul(out=t1, in0=t1, in1=var)
            nc.vector.tensor_scalar(out=t1, in0=t1, scalar1=-0.5, scalar2=1.5,
                                    op0=A.mult, op1=A.add)
            nc.vector.tensor_mul(out=y, in0=y, in1=t1)

        out_sb = pc.tile([P, NCHUNK, D], F32, name=f"osb{c}", tag="osb")
        for i in range(NCHUNK):
            nc.vector.tensor_scalar(
                out=out_sb[:, i, :], in0=t_all[:, i, :],
                scalar1=mean[:, i:i + 1], scalar2=y[:, i:i + 1],
                op0=A.subtract, op1=A.mult)
        if apply_affine:
            for i in range(NCHUNK):
                nc.vector.tensor_mul(out=out_sb[:, i, :], in0=out_sb[:, i, :],
                                     in1=gam_tile)
                nc.vector.tensor_add(out=out_sb[:, i, :], in0=out_sb[:, i, :],
                                     in1=bet_tile)
        nc.gpsimd.dma_start(
            out=out_d[c].rearrange("(i p) d -> p i d", p=P), in_=out_sb)

    # ---------------- software-pipelined channel loop ----------------
    phase1(0)
    for c in range(C + 1):
        if c < C:
            st[c]["ctx_ps"] = ctx_psum.tile([D, N], F32, name=f"ctx{c}",
                                            tag="ctx")
        for i in range(NCHUNK):
            if c < C:
                scores_exp(c, i)
            if c > 0:
                ctx_mm(c - 1, i)
            if i == 1 and c >= 2:
                # deferred fc/LN of channel c-2: lands after the first
                # scores of this channel so ScalarE never starves
                tail_b(c - 2)
            if i == 3 and c + 1 < C:
                # prefetch next channel's loads/transposes/projections
                phase1(c + 1)
        if c > 0:
            tail_a(c - 1)
    tail_b(C - 1)


def _build(apply_affine):
    nc = bacc.Bacc("TRN2", target_bir_lowering=False, debug=False, num_devices=B)
    with tile.TileContext(nc) as tc, ExitStack() as ctx:
        _emit(nc, tc, ctx, apply_affine)
    nc.compile()
    return nc


def kernel(input_Q, input_K, input_V, W_Q, W_K, W_V, W_fc, ln_gamma, ln_beta):
    input_Q = np.ascontiguousarray(np.asarray(input_Q, dtype=np.float32))
    input_K = np.ascontiguousarray(np.asarray(input_K, dtype=np.float32))
    input_V = np.ascontiguousarray(np.asarray(input_V, dtype=np.float32))
    W_Q = np.ascontiguousarray(np.asarray(W_Q, dtype=np.float32))
    W_K = np.ascontiguousarray(np.asarray(W_K, dtype=np.float32))
    W_V = np.ascontiguousarray(np.asarray(W_V, dtype=np.float32))
    W_fc = np.ascontiguousarray(np.asarray(W_fc, dtype=np.float32))
    ln_gamma = np.ascontiguousarray(np.asarray(ln_gamma, dtype=np.float32))
    ln_beta = np.ascontiguousarray(np.asarray(ln_beta, dtype=np.float32))

    apply_affine = not (np.all(ln_gamma == 1.0) and np.all(ln_beta == 0.0))

    key = ("nc", apply_affine)
    if key not in _CACHE:
        _CACHE[key] = _build(apply_affine)
    nc = _CACHE[key]

    in_maps = [
        {
            "xq": input_Q[b], "xk": input_K[b], "xv": input_V[b],
            "wq": W_Q, "wk": W_K, "wv": W_V, "wfc": W_fc,
            "gam": ln_gamma, "bet": ln_beta,
        }
        for b in range(B)
    ]
    res = run_bass_kernel_spmd(nc, in_maps, core_ids=list(range(B)))
    return np.stack([res.results[b]["out"] for b in range(B)], axis=0)
